# revision 3
# baseline (speedup 1.0000x reference)
"""Trainium2 Bass kernel for nn_CPAMDec_Mix (dual cross-attention, CPAM decoder).

Math (per batch element n):
    q_i = wq_i @ x_i + bq_i              # (D, HW)   1x1 conv query
    k_i = y_i @ wk_i.T + bk_i            # (K, D)    linear key
    v_i = y_i @ wv_i.T + bv_i            # (K, C)    linear value
    e   = | k_1 q_1 - k_2 q_2 |          # (K, HW)   (transposed layout)
    a   = softmax_K(e)
    out_i = scale * (v_i.T @ a) + x_i    # (C, HW)

Sharding: pure data parallel, one batch element per NeuronCore (N=8, 8 cores).

v6 design notes (trace-driven, see v4/v5):
  * bf16 HBM I/O, packed [128, NT*4*L] x/out layouts, 1 DMA per stream
    per tile, packed consts (3 DMAs).
  * 4/5-deep software pipeline; V-setup matmuls are EMITTED after step 1
    so the in-order PE reaches Q(0) without waiting for the wv DMA.
  * E in bf16 (fp8 q/k moved the scale=1 error to 7e-2: |logits| reach
    ~20, so relative q/k error becomes large absolute logit error).
  * softmax denominator: recip on DVE, bf16 copy on DVE (2x mode),
    partition-broadcast on GPSIMD (replaces the ones-matmul), attn
    multiply all-bf16 on DVE (2x mode).  GPSIMD runs ONLY pb: stores
    ride the SP ring (loads have drained by the time stores begin), so
    pb is never HOL-blocked behind a store's semaphore wait (v5 bug:
    12us attnmul stalls).
  * U matmuls processed in tile PAIRS so each V-chunk stationary serves
    two matmuls (weight switch costs ~115ns extra; reuse hits 216ns);
    drains grouped per chunk to bound PSUM-buffer rotation.
  * PSUM (8 banks): qp 2 | ep 1 | sp 1 | up 3 | setup 1.
"""

import numpy as np

N, C, H, W, K = 8, 512, 64, 64, 64
HW = H * W          # 4096
D = C // 4          # 128
L = 512             # pixel tile size
NT = HW // L        # 8 tiles
NCH = C // 128      # 4 contraction chunks
P = 128

# residual-drain engine per chunk j per stream:
# 'v' = DVE tensor_add, 'a' = Act cast with identity-matmul accumulation
RESID_ENG = {0: ['v', 'a', 'v', 'v'], 1: ['v', 'a', 'a', 'v']}

_SEG = {}
_off = 0
for _name, _w in (("y1", NCH * K), ("y2", NCH * K), ("wk1", NCH * D),
                  ("wk2", NCH * D), ("wq1", NCH * D), ("wq2", NCH * D),
                  ("ident", P), ("ones", K)):
    _SEG[_name] = (_off, _w)
    _off += _w
CPW = _off
WVW = 2 * C + 2 * NCH * C  # [bv1|bv2 on row 0] ++ wv1 chunks ++ wv2 chunks
FPW = 5  # f32 pack: bq1, bq2, bk1, bk2, scol

_CACHE = {}


def _build():
    from contextlib import ExitStack

    import concourse.tile as tile
    from concourse import bacc, mybir

    f32 = mybir.dt.float32
    bf16 = mybir.dt.bfloat16
    AF = mybir.ActivationFunctionType

    nc = bacc.Bacc("TRN2", target_bir_lowering=False, debug=False)

    def din(name, shape, dt=f32):
        return nc.dram_tensor(name, shape, dt, kind="ExternalInput").ap()

    def dout(name, shape, dt=bf16):
        return nc.dram_tensor(name, shape, dt, kind="ExternalOutput").ap()

    x1 = din("x1", [P, NT * NCH * L], bf16)
    x2 = din("x2", [P, NT * NCH * L], bf16)
    cpk = din("cpk", [P, CPW], bf16)
    fpk = din("fpk", [P, FPW], f32)
    wvp = din("wvp", [P, WVW], bf16)
    o1 = dout("o1", [P, NT * NCH * L])
    o2 = dout("o2", [P, NT * NCH * L])

    with tile.TileContext(nc) as tc, ExitStack() as ctx:
        cpool = ctx.enter_context(tc.tile_pool(name="const", bufs=1))

        # --- constants: packed DMAs; only x loads ride SP before wvp -------
        cps = cpool.tile([P, CPW], bf16, name="cps", tag="cps")
        nc.scalar.dma_start(cps[:], cpk[:])
        fps = cpool.tile([P, FPW], f32, name="fps", tag="fps")
        nc.scalar.dma_start(fps[:], fpk[:])
        wvs = cpool.tile([P, WVW], bf16, name="wvs", tag="wvs")

        def seg(name):
            o, w = _SEG[name]
            return cps[:, o:o + w]

        y1s, y2s = seg("y1"), seg("y2")
        wk1s, wk2s = seg("wk1"), seg("wk2")
        wq1s, wq2s = seg("wq1"), seg("wq2")
        idents = seg("ident")
        o_on, _ = _SEG["ones"]
        onrs = cps[0:1, o_on:o_on + K]  # [1,K] ones row
        oncs = cps[0:K, o_on:o_on + 1]  # [K,1] ones column
        bq1s = fps[0:D, 0:1]
        bq2s = fps[0:D, 1:2]
        bk1s = fps[0:D, 2:3]
        bk2s = fps[0:D, 3:4]
        scols = fps[0:K, 4:5]

        bk2n = cpool.tile([D, 1], f32, name="bk2n", tag="bk2n")
        nc.scalar.mul(bk2n[:], bk2s, -1.0)

        # key pack [D, 2K]: cols 0:K = k1, K:2K = -(k2+bk2)
        k12 = cpool.tile([D, 2 * K], bf16, name="k12", tag="k12")
        v1s = cpool.tile([K, C], bf16, name="v1s", tag="v1s")
        v2s = cpool.tile([K, C], bf16, name="v2s", tag="v2s")
        bvrow = cpool.tile([1, 2 * C], bf16, name="bvrow", tag="bvrow")

        # --- streaming pools ----------------------------------------------
        xpool = ctx.enter_context(tc.tile_pool(name="xpool", bufs=8))
        qsb = ctx.enter_context(tc.tile_pool(name="qsb", bufs=2))
        softp = ctx.enter_context(tc.tile_pool(name="softp", bufs=3))
        opool = ctx.enter_context(tc.tile_pool(name="opool", bufs=3))
        qpp = ctx.enter_context(tc.tile_pool(name="qpp", bufs=1, space="PSUM"))
        epp = ctx.enter_context(tc.tile_pool(name="epp", bufs=1, space="PSUM"))
        spp = ctx.enter_context(tc.tile_pool(name="spp", bufs=1, space="PSUM"))
        upp = ctx.enter_context(tc.tile_pool(name="upp", bufs=3, space="PSUM"))
        stpp = ctx.enter_context(tc.tile_pool(name="stpp", bufs=1,
                                              space="PSUM"))
        setp = stpp.tile([P, L], f32, name="setp", tag="setp")

        # --- K setup (needed by E(0) at step 1): uses setp bank ------------
        for (wks, ys, cofs, bias, sc) in (
                (wk1s, y1s, 0, bk1s, 1.0),
                (wk2s, y2s, K, bk2n[:], -1.0)):
            kp = setp[0:D, cofs:cofs + K]
            for j in range(NCH):
                nc.tensor.matmul(
                    kp, wks[:, j * D:(j + 1) * D],
                    ys[:, j * K:(j + 1) * K],
                    start=(j == 0), stop=(j == NCH - 1))
            nc.scalar.activation(k12[:, cofs:cofs + K], kp, AF.Identity,
                                 bias=bias, scale=sc)

        def setup_v():
            # bv1|bv2 live on partition row 0 of the wv pack's first 2C cols
            nc.scalar.copy(bvrow[:], wvs[0:1, 0:2 * C])
            for (ys, wvss, bvofs, vs) in (
                    (y1s, wvs[:, 2 * C:2 * C + NCH * C], 0, v1s),
                    (y2s, wvs[:, 2 * C + NCH * C:WVW], C, v2s)):
                vp = setp[0:K, :]
                for j in range(NCH):
                    nc.tensor.matmul(
                        vp, ys[:, j * K:(j + 1) * K],
                        wvss[:, j * C:(j + 1) * C],
                        start=(j == 0), stop=False)
                nc.tensor.matmul(vp, onrs, bvrow[:, bvofs:bvofs + C],
                                 start=False, stop=True)
                # fold runtime scale into V so the residual is a plain add
                nc.scalar.activation(vs[:], vp, AF.Identity, scale=scols)

        xts = {}    # t -> {s: tile}
        qs = {}     # t -> q12 tile [D, 2L] fp8
        expes = {}  # t -> expe tile
        rsbs = {}   # t -> rsb tile
        attns = {}  # t -> attn tile

        def stage_load(t):
            c0 = t * NCH * L
            xts[t] = {}
            for s, xr in ((0, x1), (1, x2)):
                xt = xpool.tile([P, NCH * L], bf16, name=f"x{s}", tag=f"x{s}")
                nc.sync.dma_start(xt[:], xr[:, c0:c0 + NCH * L])
                xts[t][s] = xt

        def stage_q(t):
            q12 = qsb.tile([D, 2 * L], bf16, name="q12", tag="q12")
            for s, (wqss, bqs) in enumerate(((wq1s, bq1s), (wq2s, bq2s))):
                qp = qpp.tile([D, L], f32, name=f"q{s}p", tag=f"q{s}p")
                for j in range(NCH):
                    nc.tensor.matmul(
                        qp[:],
                        wqss[:, j * D:(j + 1) * D],
                        xts[t][s][:, j * L:(j + 1) * L],
                        start=(j == 0), stop=(j == NCH - 1))
                nc.scalar.activation(q12[:, s * L:(s + 1) * L], qp[:],
                                     AF.Identity, bias=bqs)
            qs[t] = q12

        def stage_e(t):
            ep = epp.tile([K, L], f32, name="ep", tag="ep")
            nc.tensor.matmul(ep[:], k12[:, 0:K], qs[t][:, 0:L],
                             start=True, stop=False)
            nc.tensor.matmul(ep[:], k12[:, K:2 * K], qs[t][:, L:2 * L],
                             start=False, stop=True)
            del qs[t]
            aabs = softp.tile([K, L], f32, name="aabs", tag="aabs")
            nc.scalar.activation(aabs[:], ep[:], AF.Abs)
            expe = softp.tile([K, L], bf16, name="expe", tag="expe")
            nc.scalar.activation(expe[:], aabs[:], AF.Exp)
            expes[t] = expe

        def stage_sum(t):
            sp = spp.tile([1, L], f32, name="sp", tag="sp")
            nc.tensor.matmul(sp[:], oncs, expes[t][:], start=True, stop=True)
            rs = softp.tile([1, L], f32, name="rs", tag="rs")
            # 1/S at ~18 bits; S in [K, K*exp(~20)] so no edge cases
            nc.vector.reciprocal_approx_fast(rs[:], sp[:])
            rsb = softp.tile([1, L], bf16, name="rsb", tag="rsb")
            nc.vector.tensor_copy(rsb[:], rs[:])
            rsbs[t] = rsb

        def stage_bcast(t):
            rbp = softp.tile([K, L], bf16, name="rbp", tag="rbp")
            nc.gpsimd.partition_broadcast(rbp[:], rsbs[t][:])
            del rsbs[t]
            attn = softp.tile([K, L], bf16, name="attn", tag="attn")
            nc.vector.tensor_mul(attn[:], expes[t][:], rbp[:])
            del expes[t]
            attns[t] = attn

        def stage_out(t):
            attn = attns.pop(t)
            for s, (vs, odr) in enumerate(((v1s, o1), (v2s, o2))):
                ot = opool.tile([P, NCH * L], bf16, name=f"ot{s}",
                                tag=f"ot{s}")
                ups = {}
                for j in range(NCH):
                    acc = RESID_ENG[s][j] == 'a'
                    up = upp.tile([P, L], f32, name="up", tag="up")
                    nc.tensor.matmul(up[:], vs[:, j * P:(j + 1) * P],
                                     attn[:], start=True, stop=not acc)
                    ups[j] = up
                    if not acc:
                        nc.vector.tensor_add(ot[:, j * L:(j + 1) * L], up[:],
                                             xts[t][s][:, j * L:(j + 1) * L])
                # identity matmuls batched: one stationary load
                for j in range(NCH):
                    if RESID_ENG[s][j] == 'a':
                        nc.tensor.matmul(ups[j][:], idents,
                                         xts[t][s][:, j * L:(j + 1) * L],
                                         start=False, stop=True)
                for j in range(NCH):
                    if RESID_ENG[s][j] == 'a':
                        nc.scalar.activation(ot[:, j * L:(j + 1) * L],
                                             ups[j][:], AF.Identity)
                c0 = t * NCH * L
                nc.sync.dma_start(odr[:, c0:c0 + NCH * L], ot[:])
            del xts[t]

        PRE = 3  # x loads issued this many steps ahead of stage_q
        for step in range(NT + 4):
            if step == 0:
                for tt in range(min(PRE, NT)):
                    stage_load(tt)
                # wv pack load rides SP *after* the prologue x loads
                nc.sync.dma_start(wvs[:], wvp[:])
            if step + PRE < NT:
                stage_load(step + PRE)
            if step < NT:
                stage_q(step)
            if 0 <= step - 1 < NT:
                stage_e(step - 1)
            if step == 1:
                # V setup here: PE reaches it after Q(0..1)/E(0), by which
                # time the wv DMA has landed; consumers start at step 5
                setup_v()
            if 0 <= step - 2 < NT:
                stage_sum(step - 2)
            if 0 <= step - 3 < NT:
                # bcast(t) then out(t) in the same step: the pb/attnmul
                # latency is covered by this step's Q/E/sum matmuls, which
                # precede the U matmuls in the in-order PE queue
                stage_bcast(step - 3)
                stage_out(step - 3)

    nc.compile()
    return nc


def _get_nc():
    if "nc" not in _CACHE:
        try:
            import concourse  # noqa: F401
        except ImportError:
            import sys
            sys.path.insert(0, "/opt/trn_rl_repo")
        _CACHE["nc"] = _build()
    return _CACHE["nc"]


def _bf16_np():
    import ml_dtypes
    return ml_dtypes.bfloat16


def _pack_x(x):
    """[C, HW] f32 -> [128, NT*4*512] bf16 packed pixel-tile-major."""
    bf = _bf16_np()
    v = x.reshape(NCH, P, NT, L).transpose(1, 2, 0, 3)
    return np.ascontiguousarray(v.astype(bf)).reshape(P, NT * NCH * L)


def _unpack_o(o):
    """[128, NT*4*512] bf16 -> [C, HW] f32."""
    v = o.reshape(P, NT, NCH, L).transpose(2, 0, 1, 3)
    return np.ascontiguousarray(v, dtype=np.float32).reshape(C, HW)


def _chunked(a):
    """[C, W] -> [128, NCH*W] with chunks side by side."""
    cw = a.shape[1]
    return np.ascontiguousarray(
        a.reshape(NCH, P, cw).transpose(1, 0, 2)).reshape(P, NCH * cw)


def _make_in_maps(inputs):
    def f32(a):
        return np.ascontiguousarray(np.asarray(a, dtype=np.float32))

    bf = _bf16_np()

    def b16(a):
        return np.ascontiguousarray(np.asarray(a).astype(bf))

    x1 = np.asarray(inputs["x1"], dtype=np.float32).reshape(N, C, HW)
    x2 = np.asarray(inputs["x2"], dtype=np.float32).reshape(N, C, HW)
    y1 = np.asarray(inputs["y1"])
    y2 = np.asarray(inputs["y2"])

    scale = float(np.asarray(inputs["scale"]).reshape(-1)[0])

    fpk = np.zeros((P, FPW), np.float32)
    fpk[0:D, 0] = f32(inputs["bq1"]).reshape(-1)
    fpk[0:D, 1] = f32(inputs["bq2"]).reshape(-1)
    fpk[0:D, 2] = f32(inputs["bk1"]).reshape(-1)
    fpk[0:D, 3] = f32(inputs["bk2"]).reshape(-1)
    fpk[0:K, 4] = scale

    wv1t = b16(np.asarray(inputs["wv1"]).T)
    wv2t = b16(np.asarray(inputs["wv2"]).T)
    wvp = np.zeros((P, WVW), bf)
    wvp[0, 0:C] = b16(np.asarray(inputs["bv1"]).reshape(-1))
    wvp[0, C:2 * C] = b16(np.asarray(inputs["bv2"]).reshape(-1))
    wvp[:, 2 * C:2 * C + NCH * C] = _chunked(wv1t)
    wvp[:, 2 * C + NCH * C:] = _chunked(wv2t)

    cpk = np.zeros((P, CPW), bf)

    def put(name, arr):
        o, w = _SEG[name]
        cpk[:, o:o + w] = arr

    put("wk1", _chunked(b16(np.asarray(inputs["wk1"]).T)))
    put("wk2", _chunked(b16(np.asarray(inputs["wk2"]).T)))
    put("wq1", _chunked(b16(np.asarray(inputs["wq1"]).T)))
    put("wq2", _chunked(b16(np.asarray(inputs["wq2"]).T)))
    put("ident", np.eye(P, dtype=np.float32).astype(bf))
    o_on, w_on = _SEG["ones"]
    cpk[:, o_on:o_on + w_on] = 1.0

    in_maps = []
    for i in range(N):
        cp = cpk.copy()
        o, w = _SEG["y1"]
        cp[:, o:o + w] = _chunked(b16(y1[i].T))
        o, w = _SEG["y2"]
        cp[:, o:o + w] = _chunked(b16(y2[i].T))
        in_maps.append({
            "cpk": cp,
            "fpk": fpk,
            "wvp": wvp,
            "x1": _pack_x(x1[i]),
            "x2": _pack_x(x2[i]),
        })
    return in_maps


def kernel(**inputs):
    nc = _get_nc()
    from concourse.bass_utils import run_bass_kernel_spmd

    in_maps = _make_in_maps(inputs)
    res = run_bass_kernel_spmd(nc, in_maps, list(range(N))).results
    out1 = np.stack([_unpack_o(res[i]["o1"]) for i in range(N)])
    out2 = np.stack([_unpack_o(res[i]["o2"]) for i in range(N)])
    return out1.reshape(N, C, H, W), out2.reshape(N, C, H, W)
